# revision 21
# baseline (speedup 1.0000x reference)
"""Trainium2 Bass kernel for sparse_attention (nn_Attention_69965017252614).

Strategy: shard the 2048 query positions across 8 NeuronCores (256 each).
Heavy O(L^2) work stays on device; tiny O(L*F) coefficient tables are
precomputed on host and passed as extra inputs.

Math (per batch, per head h):
  scores = (q/8) @ k^T
  dm[q,k,f<3] = d_q[q,f]*A[k,f] - Bv[k,f]         (rank-4 via augmented matmul)
  dm[q,k,3]   = R[q,k] = rel_top*g0[k] + rel_bot*g1[k]   (dense, head-indep)
  w_pre = dq_aug @ rhsW_h + W_w[3,h]*R ;  b_pre likewise with W_b
  s_fin = scores * softplus(w_pre) + b_pre
  probs = exp(s_fin)/rowsum * c ;  out = probs @ v

Engine assignment per (head, q-tile) row [128 x 2048]:
  PE:   QK matmuls, aug matmuls, probs transposes, PV matmuls
  DVE:  scalar_tensor_tensor (R*u + psum) for w/b, scores*w, reciprocal
  ACT:  softplus via ln(1+exp(x)) (stays in natural_log_exp table set),
        exp with accum_out (free softmax denominator), PSUM->SBUF copies
  Pool: s_fin add, final probs scale (unorm*recip)*c_bcast
"""

import sys
import numpy as np

for _p in ("/opt/trn_rl_repo", "/opt/pypackages"):
    if _p not in sys.path:
        sys.path.insert(0, _p)

B, H, L, D = 1, 8, 2048, 64
NCORES = 8
LQ = L // NCORES          # 256 queries per core
NROW = LQ // 128          # 2 q-tiles of 128 per core
KC = 1024                 # k-chunk for w/b psum tiles

_PROGRAM = None           # (nc, meta) cache — compile once per process
LAST_RESULTS = None       # BassKernelResults from the last run (for test.py)


def _patch_act_tables(bacc, mybir):
    """Make natural_log_exp_and_others the only table set advertising Exp/Ln
    so the act-table-load pass never alternates sets between the softplus
    (exp+ln) and softmax (exp) activations. Set order (= act_func_set_id)
    is preserved; only membership is edited, so the loaded table is still
    the right one."""
    if getattr(bacc, "_act_tables_patched", False):
        return
    orig = bacc.get_activation_tables
    AF = mybir.ActivationFunctionType
    keep = "natural_log_exp_and_others"

    def patched(arch):
        tabs = orig(arch)
        if keep in tabs:
            for name, fns in tabs.items():
                if name != keep:
                    fns.discard(AF.Exp)
                    fns.discard(AF.Ln)
        return tabs

    bacc.get_activation_tables = patched
    bacc._act_tables_patched = True


def _build_program():
    import concourse.bacc as bacc
    import concourse.mybir as mybir
    from concourse.tile import TileContext

    f32 = mybir.dt.float32
    f32r = mybir.dt.float32r
    AF = mybir.ActivationFunctionType
    OP = mybir.AluOpType

    _patch_act_tables(bacc, mybir)
    nc = bacc.Bacc("TRN2", target_bir_lowering=False, debug=False)

    t_qT = nc.dram_tensor("qt_in", (64, H * LQ), f32r, kind="ExternalInput").ap()
    t_kT = nc.dram_tensor("kt_in", (4, 64, 2 * L), f32r, kind="ExternalInput").ap()
    t_v = nc.dram_tensor("v_in", (4, 128, 2 * (L // 128) * D), f32r, kind="ExternalInput").ap()
    t_R = nc.dram_tensor("r_in", (128, NROW * L), f32r, kind="ExternalInput").ap()
    t_rww = nc.dram_tensor("rww_in", (H, 4, L), f32r, kind="ExternalInput").ap()
    t_rwbb = nc.dram_tensor("rwb_in", (H, 4, L), f32r, kind="ExternalInput").ap()
    t_dqT = nc.dram_tensor("dqt_in", (4, LQ), f32r, kind="ExternalInput").ap()
    t_uw = nc.dram_tensor("uw_in", (128, H), f32, kind="ExternalInput").ap()
    t_ub = nc.dram_tensor("ub_in", (128, H), f32, kind="ExternalInput").ap()
    t_cb = nc.dram_tensor("cb_in", (128, L), f32, kind="ExternalInput").ap()
    t_id = nc.dram_tensor("id_in", (128, 128), f32, kind="ExternalInput").ap()

    o_probs = nc.dram_tensor("probs_out", (H, LQ, L), f32, kind="ExternalOutput").ap()
    o_out = nc.dram_tensor("out_out", (H, D, LQ), f32, kind="ExternalOutput").ap()

    VH = (L // 128) * D   # 1024 floats of v per head per partition

    with TileContext(nc) as tc:
        with (
            tc.tile_pool(name="static", bufs=1) as st_pool,
            tc.tile_pool(name="kt", bufs=2) as kt_pool,
            tc.tile_pool(name="vp", bufs=2) as v_pool,
            tc.tile_pool(name="rwb", bufs=1) as rwb_pool,
            tc.tile_pool(name="work", bufs=5) as work_pool,
            tc.tile_pool(name="probs", bufs=4) as pr_pool,
            tc.tile_pool(name="pt", bufs=2) as pt_pool,
            tc.tile_pool(name="small", bufs=4) as sm_pool,
            tc.tile_pool(name="ps_s", bufs=1, space="PSUM") as ps_s_pool,
            tc.tile_pool(name="ps_w", bufs=1, space="PSUM") as ps_w_pool,
            tc.tile_pool(name="ps_b", bufs=1, space="PSUM") as ps_b_pool,
            tc.tile_pool(name="ps_t", bufs=2, space="PSUM") as ps_t_pool,
        ):
            # ---- static loads, ordered so the first head can start ASAP ------
            qT_sb = st_pool.tile([64, H * LQ], f32r, tag="qT")
            nc.sync.dma_start(qT_sb[:], t_qT[:])
            kt_tiles = {}
            v_tiles = {}
            kt0 = kt_pool.tile([64, L], f32r, tag="kt")
            nc.sync.dma_start(kt0[:], t_kT[0, :, 0:L])
            kt_tiles[0] = kt0
            dqT_sb = st_pool.tile([4, LQ], f32r, tag="dqT")
            nc.sync.dma_start(dqT_sb[:], t_dqT[:])
            uw_sb = st_pool.tile([128, H], f32, tag="uw")
            nc.sync.dma_start(uw_sb[:], t_uw[:])
            ub_sb = st_pool.tile([128, H], f32, tag="ub")
            nc.sync.dma_start(ub_sb[:], t_ub[:])
            id_sb = st_pool.tile([128, 128], f32, tag="ident")
            nc.sync.dma_start(id_sb[:], t_id[:])
            rww0 = rwb_pool.tile([4, L], f32r, tag="rww")
            nc.sync.dma_start(rww0[:], t_rww[0])
            rwbb0 = rwb_pool.tile([4, L], f32r, tag="rwbb")
            nc.sync.dma_start(rwbb0[:], t_rwbb[0])
            R_sb = st_pool.tile([128, NROW * L], f32r, tag="R")
            nc.sync.dma_start(R_sb[:, 0:L], t_R[:, 0:L])
            nc.sync.dma_start(R_sb[:, L : 2 * L], t_R[:, L : 2 * L])
            cb_sb = st_pool.tile([128, L], f32, tag="cb")
            nc.sync.dma_start(cb_sb[:], t_cb[:])

            # deferred-stage state from the previous head
            pend = {}

            def emit_head_tail(ph):
                """Transposes + PV + output stores for head ph (deferred)."""
                p_off = ph % 2
                pv_t = pend["v_t"]
                probsT_h = pt_pool.tile([128, (L // 128) * 256], f32r, tag="pT")
                for qt in range(NROW):
                    probs = pend[("probs", qt)]
                    for g in range(4):
                        ps_t = ps_t_pool.tile([128, 512], f32, tag="T")
                        for j in range(4):
                            kb = g * 4 + j
                            nc.tensor.transpose(
                                ps_t[:, j * 128 : (j + 1) * 128],
                                probs[:, kb * 128 : (kb + 1) * 128],
                                id_sb[:],
                            )
                        dst = probsT_h.rearrange("p (kb q) -> p kb q", q=256)[
                            :, g * 4 : (g + 1) * 4, qt * 128 : qt * 128 + 128]
                        src_ap = ps_t[:].rearrange("p (kb q) -> p kb q", q=128)
                        if g % 2 == 0:
                            nc.scalar.copy(dst, src_ap)
                        else:
                            nc.vector.tensor_copy(dst, src_ap)
                ps_oT = ps_t_pool.tile([64, 2 * 128], f32, tag="T")
                nkb = L // 128
                for kb in range(nkb):
                    nc.tensor.matmul(
                        ps_oT[:],
                        pv_t[:, p_off * VH + kb * D : p_off * VH + (kb + 1) * D],
                        probsT_h[:, kb * 256 : (kb + 1) * 256],
                        start=(kb == 0), stop=(kb == nkb - 1),
                    )
                oT_sb = sm_pool.tile([64, 2 * 128], f32, tag="oT")
                nc.scalar.copy(oT_sb[:], ps_oT[:])
                nc.sync.dma_start(o_out[ph], oT_sb[:])

            for h in range(H):
                pair, off = h // 2, h % 2
                if h + 1 < H and (h + 1) // 2 not in kt_tiles:
                    kt_new = kt_pool.tile([64, L], f32r, tag="kt")
                    nc.sync.dma_start(kt_new[:], t_kT[(h + 1) // 2, :, 0:L])
                    kt_tiles[(h + 1) // 2] = kt_new
                if pair not in v_tiles:
                    v_new = v_pool.tile([128, 2 * VH], f32r, tag="vp")
                    nc.sync.dma_start(v_new[:], t_v[pair])
                    v_tiles[pair] = v_new
                # second half of the pair's kT arrives with the odd head
                if off == 0:
                    kt_odd = kt_pool.tile([64, L], f32r, tag="kt")
                    nc.sync.dma_start(kt_odd[:], t_kT[pair, :, L : 2 * L])
                    kt_tiles[pair, 1] = kt_odd
                kt_t = kt_tiles[pair] if off == 0 else kt_tiles[pair, 1]
                v_t = v_tiles[pair]

                uwI = sm_pool.tile([128, 128], f32r, tag="uwI")
                nc.vector.tensor_scalar_mul(uwI[:], id_sb[:], uw_sb[:, h : h + 1])
                ubI = sm_pool.tile([128, 128], f32r, tag="ubI")
                nc.vector.tensor_scalar_mul(ubI[:], id_sb[:], ub_sb[:, h : h + 1])
                if h == 0:
                    rww_t, rwbb_t = rww0, rwbb0
                else:
                    rww_t = rwb_pool.tile([4, L], f32r, tag="rww")
                    nc.sync.dma_start(rww_t[:], t_rww[h])
                    rwbb_t = rwb_pool.tile([4, L], f32r, tag="rwbb")
                    nc.sync.dma_start(rwbb_t[:], t_rwbb[h])

                new_pend = {"v_t": v_t}
                for qt in range(NROW):
                    qsl = qT_sb[:, h * LQ + qt * 128 : h * LQ + qt * 128 + 128]
                    dqsl = dqT_sb[:, qt * 128 : qt * 128 + 128]
                    Rrow = R_sb[:, qt * L : (qt + 1) * L]

                    # --- scores: two [128, 1024] psum tiles -------------------
                    ps_s_half = []
                    for cs in range(2):
                        ps_s = ps_s_pool.tile([128, 1024], f32, tag="s")
                        ps_s_half.append(ps_s)
                        for n in range(2):
                            col = cs * 1024 + n * 512
                            nc.tensor.matmul(
                                ps_s[:, n * 512 : (n + 1) * 512],
                                qsl,
                                kt_t[:, col : col + 512],
                                start=True,
                                stop=True,
                            )

                    # --- w/b aug matmuls; u*R accumulated via f32r identity ---
                    w_row = work_pool.tile([128, L], f32, tag="work")
                    sw_row = work_pool.tile([128, L], f32, tag="work")
                    sfin = work_pool.tile([128, L], f32, tag="work")
                    denom = sm_pool.tile([128, 2], f32, tag="denom")
                    for cs in range(2):
                        sl2 = slice(cs * 1024, (cs + 1) * 1024)
                        ps_w = ps_w_pool.tile([128, 1024], f32, tag="w")
                        ps_b = ps_b_pool.tile([128, 1024], f32, tag="b")
                        for n in range(2):
                            ssl = slice(cs * 1024 + n * 512, cs * 1024 + (n + 1) * 512)
                            nsl = slice(n * 512, (n + 1) * 512)
                            nc.tensor.matmul(ps_w[:, nsl], dqsl, rww_t[:, ssl],
                                             start=True, stop=False)
                            nc.tensor.matmul(ps_w[:, nsl], uwI[:], Rrow[:, ssl],
                                             start=False, stop=True)
                            nc.tensor.matmul(ps_b[:, nsl], dqsl, rwbb_t[:, ssl],
                                             start=True, stop=False)
                            nc.tensor.matmul(ps_b[:, nsl], ubI[:], Rrow[:, ssl],
                                             start=False, stop=True)
                        # softplus from psum: w = ln(1 + exp(ps_w))
                        nc.scalar.activation(w_row[:, sl2], ps_w[:], AF.Exp)
                        nc.scalar.activation(w_row[:, sl2], w_row[:, sl2], AF.Ln, bias=1.0)
                        nc.vector.tensor_tensor(sw_row[:, sl2], ps_s_half[cs][:],
                                                w_row[:, sl2], op=OP.mult)
                        nc.vector.tensor_tensor(sfin[:, sl2], sw_row[:, sl2],
                                                ps_b[:], op=OP.add)
                        nc.scalar.activation(sfin[:, sl2], sfin[:, sl2], AF.Exp,
                                             accum_out=denom[:, cs : cs + 1])
                    unorm = sfin
                    recip = sm_pool.tile([128, 1], f32, tag="recip")
                    dsum = sm_pool.tile([128, 1], f32, tag="dsum")
                    nc.vector.tensor_tensor(dsum[:], denom[:, 0:1], denom[:, 1:2], op=OP.add)
                    nc.vector.reciprocal(recip[:], dsum[:])

                    # --- probs: t2 = unorm*c (Pool), probs = t2*recip (DVE) ---
                    probs = pr_pool.tile([128, L], f32, tag="probs")
                    t2 = work_pool.tile([128, L], f32, tag="work")
                    for cs in range(2):
                        sl2 = slice(cs * 1024, (cs + 1) * 1024)
                        nc.gpsimd.tensor_tensor(t2[:, sl2], unorm[:, sl2],
                                                cb_sb[:, sl2], op=OP.mult)
                        nc.vector.tensor_scalar_mul(probs[:, sl2], t2[:, sl2], recip[:])
                    nc.sync.dma_start(o_probs[h, qt * 128 : (qt + 1) * 128, :], probs[:])
                    new_pend[("probs", qt)] = probs

                if pend:
                    emit_head_tail(h - 1)
                pend = new_pend
            emit_head_tail(H - 1)

    nc.compile()
    return nc


def _host_prep(q, k, v, c, d_q, d_k_top, d_k_bot, d_k_score,
               relative_top, relative_bottom, W_w, W_b):
    """Build the per-core input maps (all fp32 numpy)."""
    f = np.float32
    q = np.asarray(q, f); k = np.asarray(k, f); v = np.asarray(v, f)
    c = np.asarray(c, f); d_q = np.asarray(d_q, f)
    d_k_top = np.asarray(d_k_top, f); d_k_bot = np.asarray(d_k_bot, f)
    d_k_score = np.asarray(d_k_score, f)
    relative_top = np.asarray(relative_top, f)
    relative_bottom = np.asarray(relative_bottom, f)
    W_w = np.asarray(W_w, f); W_b = np.asarray(W_b, f)

    scale = f(1.0 / np.sqrt(D))

    # qT per core: [64, H*LQ], h-major columns
    qh = q[0] * scale                                   # [H, L, D]
    # kT pairs: [4, 64, 2L]
    kk = np.transpose(k[0], (2, 0, 1))                  # [64, H, L]
    kT_pack = np.ascontiguousarray(
        np.transpose(kk.reshape(64, 4, 2 * L), (1, 0, 2)))
    # v pairs: [4, 128, 2*VH]
    vv = v[0].reshape(H, L // 128, 128, D).transpose(0, 2, 1, 3).reshape(H, 128, -1)
    VH = vv.shape[2]
    v_pack = np.ascontiguousarray(
        vv.reshape(4, 2, 128, VH).transpose(0, 2, 1, 3).reshape(4, 128, 2 * VH))

    # R = rel_top*g0[k] + rel_bot*g1[k]
    g0 = d_k_score[0, :, 0, 3]
    g1 = d_k_score[0, :, 1, 3]
    R_full = relative_top[0, :, :, 0] * g0[None, :] + relative_bottom[0, :, :, 0] * g1[None, :]

    # coefficient tables
    s0 = d_k_score[0, :, 0, :]                          # [L, 4]
    s1 = d_k_score[0, :, 1, :]
    A3 = (s0 + s1)[:, :3]                               # [L, 3]
    Bv = d_k_top[0] * s0[:, :3] + d_k_bot[0] * s1[:, :3]
    rww = np.empty((H, 4, L), f)
    rwbb = np.empty((H, 4, L), f)
    for h in range(H):
        for fdim in range(3):
            rww[h, fdim] = A3[:, fdim] * W_w[fdim, h]
            rwbb[h, fdim] = A3[:, fdim] * W_b[fdim, h]
        rww[h, 3] = -(Bv @ W_w[:3, h])
        rwbb[h, 3] = -(Bv @ W_b[:3, h])

    uw = np.broadcast_to(W_w[3], (128, H)).copy()
    ub = np.broadcast_to(W_b[3], (128, H)).copy()
    cb = np.broadcast_to(c[0, 0, 0], (128, L)).copy()
    ident = np.eye(128, dtype=f)

    in_maps = []
    for core in range(NCORES):
        qs = qh[:, core * LQ : (core + 1) * LQ, :]      # [H, LQ, D]
        qT = np.ascontiguousarray(np.transpose(qs, (2, 0, 1)).reshape(64, H * LQ))
        Rs = R_full[core * LQ : (core + 1) * LQ]
        Rp = np.ascontiguousarray(Rs.reshape(NROW, 128, L).transpose(1, 0, 2).reshape(128, NROW * L))
        dqT = np.concatenate(
            [d_q[0, core * LQ : (core + 1) * LQ].T, np.ones((1, LQ), f)], axis=0)
        in_maps.append({
            "qt_in": qT, "kt_in": kT_pack, "v_in": v_pack, "r_in": Rp,
            "rww_in": rww, "rwb_in": rwbb, "dqt_in": np.ascontiguousarray(dqT),
            "uw_in": uw, "ub_in": ub, "cb_in": cb, "id_in": ident,
        })
    return in_maps


def kernel(q, k, v, c, d_q, d_k_top, d_k_bot, d_k_score,
           relative_top, relative_bottom, W_w, W_b,
           trace=False):
    global _PROGRAM, LAST_RESULTS
    from concourse import bass_utils

    if _PROGRAM is None:
        _PROGRAM = _build_program()
    nc = _PROGRAM

    in_maps = _host_prep(q, k, v, c, d_q, d_k_top, d_k_bot, d_k_score,
                         relative_top, relative_bottom, W_w, W_b)

    res = bass_utils.run_bass_kernel_spmd(
        nc, in_maps, core_ids=list(range(NCORES)), trace=trace)
    LAST_RESULTS = res

    probs = np.empty((B, H, L, L), np.float32)
    out = np.empty((B, H, L, D), np.float32)
    for core in range(NCORES):
        r = res.results[core]
        probs[0, :, core * LQ : (core + 1) * LQ, :] = r["probs_out"]
        out[0, :, core * LQ : (core + 1) * LQ, :] = np.transpose(r["out_out"], (0, 2, 1))
    return out, probs


# revision 22
# speedup vs baseline: 1.0178x; 1.0178x over previous
"""Trainium2 Bass kernel for sparse_attention (nn_Attention_69965017252614).

Strategy: shard the 2048 query positions across 8 NeuronCores (256 each).
Heavy O(L^2) work stays on device; tiny O(L*F) coefficient tables are
precomputed on host and passed as extra inputs.

Math (per batch, per head h):
  scores = (q/8) @ k^T
  dm[q,k,f<3] = d_q[q,f]*A[k,f] - Bv[k,f]         (rank-4 via augmented matmul)
  dm[q,k,3]   = R[q,k] = rel_top*g0[k] + rel_bot*g1[k]   (dense, head-indep)
  w_pre = dq_aug @ rhsW_h + W_w[3,h]*R ;  b_pre likewise with W_b
  s_fin = scores * softplus(w_pre) + b_pre
  probs = exp(s_fin)/rowsum * c ;  out = probs @ v

Engine assignment per (head, q-tile) row [128 x 2048]:
  PE:   QK matmuls, aug matmuls, probs transposes, PV matmuls
  DVE:  scalar_tensor_tensor (R*u + psum) for w/b, scores*w, reciprocal
  ACT:  softplus via ln(1+exp(x)) (stays in natural_log_exp table set),
        exp with accum_out (free softmax denominator), PSUM->SBUF copies
  Pool: s_fin add, final probs scale (unorm*recip)*c_bcast
"""

import sys
import numpy as np

for _p in ("/opt/trn_rl_repo", "/opt/pypackages"):
    if _p not in sys.path:
        sys.path.insert(0, _p)

B, H, L, D = 1, 8, 2048, 64
NCORES = 8
LQ = L // NCORES          # 256 queries per core
NROW = LQ // 128          # 2 q-tiles of 128 per core
KC = 1024                 # k-chunk for w/b psum tiles

_PROGRAM = None           # (nc, meta) cache — compile once per process
LAST_RESULTS = None       # BassKernelResults from the last run (for test.py)


def _patch_act_tables(bacc, mybir):
    """Make natural_log_exp_and_others the only table set advertising Exp/Ln
    so the act-table-load pass never alternates sets between the softplus
    (exp+ln) and softmax (exp) activations. Set order (= act_func_set_id)
    is preserved; only membership is edited, so the loaded table is still
    the right one."""
    if getattr(bacc, "_act_tables_patched", False):
        return
    orig = bacc.get_activation_tables
    AF = mybir.ActivationFunctionType
    keep = "natural_log_exp_and_others"

    def patched(arch):
        tabs = orig(arch)
        if keep in tabs:
            for name, fns in tabs.items():
                if name != keep:
                    fns.discard(AF.Exp)
                    fns.discard(AF.Ln)
        return tabs

    bacc.get_activation_tables = patched
    bacc._act_tables_patched = True


def _build_program():
    import concourse.bacc as bacc
    import concourse.mybir as mybir
    from concourse.tile import TileContext

    f32 = mybir.dt.float32
    f32r = mybir.dt.float32r
    AF = mybir.ActivationFunctionType
    OP = mybir.AluOpType

    _patch_act_tables(bacc, mybir)
    nc = bacc.Bacc("TRN2", target_bir_lowering=False, debug=False)

    t_qT = nc.dram_tensor("qt_in", (64, H * LQ), f32r, kind="ExternalInput").ap()
    t_kT = nc.dram_tensor("kt_in", (4, 64, 2 * L), f32r, kind="ExternalInput").ap()
    t_v = nc.dram_tensor("v_in", (4, 128, 2 * (L // 128) * D), f32r, kind="ExternalInput").ap()
    t_R = nc.dram_tensor("r_in", (128, NROW * L), f32r, kind="ExternalInput").ap()
    t_rww = nc.dram_tensor("rww_in", (H, 4, L), f32r, kind="ExternalInput").ap()
    t_rwbb = nc.dram_tensor("rwb_in", (H, 4, L), f32r, kind="ExternalInput").ap()
    t_dqT = nc.dram_tensor("dqt_in", (4, LQ), f32r, kind="ExternalInput").ap()
    t_uw = nc.dram_tensor("uw_in", (128, H), f32, kind="ExternalInput").ap()
    t_ub = nc.dram_tensor("ub_in", (128, H), f32, kind="ExternalInput").ap()
    t_cb = nc.dram_tensor("cb_in", (128, L), f32, kind="ExternalInput").ap()
    t_id = nc.dram_tensor("id_in", (128, 128), f32, kind="ExternalInput").ap()

    o_probs = nc.dram_tensor("probs_out", (H, LQ, L), f32, kind="ExternalOutput").ap()
    o_out = nc.dram_tensor("out_out", (H, D, LQ), f32, kind="ExternalOutput").ap()

    VH = (L // 128) * D   # 1024 floats of v per head per partition

    with TileContext(nc) as tc:
        with (
            tc.tile_pool(name="static", bufs=1) as st_pool,
            tc.tile_pool(name="kt", bufs=2) as kt_pool,
            tc.tile_pool(name="vp", bufs=2) as v_pool,
            tc.tile_pool(name="rwb", bufs=1) as rwb_pool,
            tc.tile_pool(name="work", bufs=5) as work_pool,
            tc.tile_pool(name="probs", bufs=4) as pr_pool,
            tc.tile_pool(name="pt", bufs=2) as pt_pool,
            tc.tile_pool(name="small", bufs=4) as sm_pool,
            tc.tile_pool(name="ps_s", bufs=1, space="PSUM") as ps_s_pool,
            tc.tile_pool(name="ps_w", bufs=1, space="PSUM") as ps_w_pool,
            tc.tile_pool(name="ps_b", bufs=1, space="PSUM") as ps_b_pool,
            tc.tile_pool(name="ps_t", bufs=2, space="PSUM") as ps_t_pool,
        ):
            # warm the natural_log_exp act-table while static DMAs stream in
            warm = sm_pool.tile([128, 1], f32, tag="warm")
            nc.gpsimd.memset(warm[:], 0.0)
            nc.scalar.activation(warm[:], warm[:], AF.Exp)
            nc.scalar.activation(warm[:], warm[:], AF.Ln, bias=1.0)

            # ---- static loads, ordered so the first head can start ASAP ------
            qT_sb = st_pool.tile([64, H * LQ], f32r, tag="qT")
            nc.sync.dma_start(qT_sb[:], t_qT[:])
            kt_tiles = {}
            v_tiles = {}
            kt0 = kt_pool.tile([64, L], f32r, tag="kt")
            nc.sync.dma_start(kt0[:], t_kT[0, :, 0:L])
            kt_tiles[0] = kt0
            dqT_sb = st_pool.tile([4, LQ], f32r, tag="dqT")
            nc.sync.dma_start(dqT_sb[:], t_dqT[:])
            uw_sb = st_pool.tile([128, H], f32, tag="uw")
            nc.sync.dma_start(uw_sb[:], t_uw[:])
            ub_sb = st_pool.tile([128, H], f32, tag="ub")
            nc.sync.dma_start(ub_sb[:], t_ub[:])
            id_sb = st_pool.tile([128, 128], f32, tag="ident")
            nc.sync.dma_start(id_sb[:], t_id[:])
            rww0 = rwb_pool.tile([4, L], f32r, tag="rww")
            nc.sync.dma_start(rww0[:], t_rww[0])
            rwbb0 = rwb_pool.tile([4, L], f32r, tag="rwbb")
            nc.sync.dma_start(rwbb0[:], t_rwbb[0])
            R_sb = st_pool.tile([128, NROW * L], f32r, tag="R")
            nc.sync.dma_start(R_sb[:, 0:L], t_R[:, 0:L])
            nc.sync.dma_start(R_sb[:, L : 2 * L], t_R[:, L : 2 * L])
            cb_sb = st_pool.tile([128, L], f32, tag="cb")
            nc.sync.dma_start(cb_sb[:], t_cb[:])

            # deferred-stage state from the previous head
            pend = {}

            def emit_head_tail(ph):
                """Transposes + PV + output stores for head ph (deferred)."""
                p_off = ph % 2
                pv_t = pend["v_t"]
                probsT_h = pt_pool.tile([128, (L // 128) * 256], f32r, tag="pT")
                for qt in range(NROW):
                    probs = pend[("probs", qt)]
                    for g in range(4):
                        ps_t = ps_t_pool.tile([128, 512], f32, tag="T")
                        for j in range(4):
                            kb = g * 4 + j
                            nc.tensor.transpose(
                                ps_t[:, j * 128 : (j + 1) * 128],
                                probs[:, kb * 128 : (kb + 1) * 128],
                                id_sb[:],
                            )
                        dst = probsT_h.rearrange("p (kb q) -> p kb q", q=256)[
                            :, g * 4 : (g + 1) * 4, qt * 128 : qt * 128 + 128]
                        src_ap = ps_t[:].rearrange("p (kb q) -> p kb q", q=128)
                        if g % 2 == 0:
                            nc.scalar.copy(dst, src_ap)
                        else:
                            nc.vector.tensor_copy(dst, src_ap)
                ps_oT = ps_t_pool.tile([64, 2 * 128], f32, tag="T")
                nkb = L // 128
                for kb in range(nkb):
                    nc.tensor.matmul(
                        ps_oT[:],
                        pv_t[:, p_off * VH + kb * D : p_off * VH + (kb + 1) * D],
                        probsT_h[:, kb * 256 : (kb + 1) * 256],
                        start=(kb == 0), stop=(kb == nkb - 1),
                    )
                oT_sb = sm_pool.tile([64, 2 * 128], f32, tag="oT")
                nc.scalar.copy(oT_sb[:], ps_oT[:])
                nc.sync.dma_start(o_out[ph], oT_sb[:])

            for h in range(H):
                pair, off = h // 2, h % 2
                if h + 1 < H and (h + 1) // 2 not in kt_tiles:
                    kt_new = kt_pool.tile([64, L], f32r, tag="kt")
                    nc.sync.dma_start(kt_new[:], t_kT[(h + 1) // 2, :, 0:L])
                    kt_tiles[(h + 1) // 2] = kt_new
                if pair not in v_tiles:
                    v_new = v_pool.tile([128, 2 * VH], f32r, tag="vp")
                    nc.sync.dma_start(v_new[:], t_v[pair])
                    v_tiles[pair] = v_new
                # second half of the pair's kT arrives with the odd head
                if off == 0:
                    kt_odd = kt_pool.tile([64, L], f32r, tag="kt")
                    nc.sync.dma_start(kt_odd[:], t_kT[pair, :, L : 2 * L])
                    kt_tiles[pair, 1] = kt_odd
                kt_t = kt_tiles[pair] if off == 0 else kt_tiles[pair, 1]
                v_t = v_tiles[pair]

                uwI = sm_pool.tile([128, 128], f32r, tag="uwI")
                nc.vector.tensor_scalar_mul(uwI[:], id_sb[:], uw_sb[:, h : h + 1])
                ubI = sm_pool.tile([128, 128], f32r, tag="ubI")
                nc.vector.tensor_scalar_mul(ubI[:], id_sb[:], ub_sb[:, h : h + 1])
                if h == 0:
                    rww_t, rwbb_t = rww0, rwbb0
                else:
                    rww_t = rwb_pool.tile([4, L], f32r, tag="rww")
                    nc.sync.dma_start(rww_t[:], t_rww[h])
                    rwbb_t = rwb_pool.tile([4, L], f32r, tag="rwbb")
                    nc.sync.dma_start(rwbb_t[:], t_rwbb[h])

                new_pend = {"v_t": v_t}
                for qt in range(NROW):
                    qsl = qT_sb[:, h * LQ + qt * 128 : h * LQ + qt * 128 + 128]
                    dqsl = dqT_sb[:, qt * 128 : qt * 128 + 128]
                    Rrow = R_sb[:, qt * L : (qt + 1) * L]

                    # --- scores: two [128, 1024] psum tiles -------------------
                    ps_s_half = []
                    for cs in range(2):
                        ps_s = ps_s_pool.tile([128, 1024], f32, tag="s")
                        ps_s_half.append(ps_s)
                        for n in range(2):
                            col = cs * 1024 + n * 512
                            nc.tensor.matmul(
                                ps_s[:, n * 512 : (n + 1) * 512],
                                qsl,
                                kt_t[:, col : col + 512],
                                start=True,
                                stop=True,
                            )

                    # --- w/b aug matmuls; u*R accumulated via f32r identity ---
                    w_row = work_pool.tile([128, L], f32, tag="work")
                    sw_row = work_pool.tile([128, L], f32, tag="work")
                    sfin = work_pool.tile([128, L], f32, tag="work")
                    denom = sm_pool.tile([128, 2], f32, tag="denom")
                    for cs in range(2):
                        sl2 = slice(cs * 1024, (cs + 1) * 1024)
                        ps_w = ps_w_pool.tile([128, 1024], f32, tag="w")
                        ps_b = ps_b_pool.tile([128, 1024], f32, tag="b")
                        for n in range(2):
                            ssl = slice(cs * 1024 + n * 512, cs * 1024 + (n + 1) * 512)
                            nsl = slice(n * 512, (n + 1) * 512)
                            nc.tensor.matmul(ps_w[:, nsl], dqsl, rww_t[:, ssl],
                                             start=True, stop=False)
                            nc.tensor.matmul(ps_w[:, nsl], uwI[:], Rrow[:, ssl],
                                             start=False, stop=True)
                            nc.tensor.matmul(ps_b[:, nsl], dqsl, rwbb_t[:, ssl],
                                             start=True, stop=False)
                            nc.tensor.matmul(ps_b[:, nsl], ubI[:], Rrow[:, ssl],
                                             start=False, stop=True)
                        # softplus from psum: w = ln(1 + exp(ps_w))
                        nc.scalar.activation(w_row[:, sl2], ps_w[:], AF.Exp)
                        nc.scalar.activation(w_row[:, sl2], w_row[:, sl2], AF.Ln, bias=1.0)
                        nc.vector.tensor_tensor(sw_row[:, sl2], ps_s_half[cs][:],
                                                w_row[:, sl2], op=OP.mult)
                        nc.vector.tensor_tensor(sfin[:, sl2], sw_row[:, sl2],
                                                ps_b[:], op=OP.add)
                        nc.scalar.activation(sfin[:, sl2], sfin[:, sl2], AF.Exp,
                                             accum_out=denom[:, cs : cs + 1])
                    unorm = sfin
                    recip = sm_pool.tile([128, 1], f32, tag="recip")
                    dsum = sm_pool.tile([128, 1], f32, tag="dsum")
                    nc.vector.tensor_tensor(dsum[:], denom[:, 0:1], denom[:, 1:2], op=OP.add)
                    nc.vector.reciprocal(recip[:], dsum[:])

                    # --- probs: t2 = unorm*c (Pool), probs = t2*recip (DVE) ---
                    probs = pr_pool.tile([128, L], f32, tag="probs")
                    t2 = work_pool.tile([128, L], f32, tag="work")
                    for cs in range(2):
                        sl2 = slice(cs * 1024, (cs + 1) * 1024)
                        nc.gpsimd.tensor_tensor(t2[:, sl2], unorm[:, sl2],
                                                cb_sb[:, sl2], op=OP.mult)
                        nc.vector.tensor_scalar_mul(probs[:, sl2], t2[:, sl2], recip[:])
                    nc.sync.dma_start(o_probs[h, qt * 128 : (qt + 1) * 128, :], probs[:])
                    new_pend[("probs", qt)] = probs

                if pend:
                    emit_head_tail(h - 1)
                pend = new_pend
            emit_head_tail(H - 1)

    nc.compile()
    return nc


def _host_prep(q, k, v, c, d_q, d_k_top, d_k_bot, d_k_score,
               relative_top, relative_bottom, W_w, W_b):
    """Build the per-core input maps (all fp32 numpy)."""
    f = np.float32
    q = np.asarray(q, f); k = np.asarray(k, f); v = np.asarray(v, f)
    c = np.asarray(c, f); d_q = np.asarray(d_q, f)
    d_k_top = np.asarray(d_k_top, f); d_k_bot = np.asarray(d_k_bot, f)
    d_k_score = np.asarray(d_k_score, f)
    relative_top = np.asarray(relative_top, f)
    relative_bottom = np.asarray(relative_bottom, f)
    W_w = np.asarray(W_w, f); W_b = np.asarray(W_b, f)

    scale = f(1.0 / np.sqrt(D))

    # qT per core: [64, H*LQ], h-major columns
    qh = q[0] * scale                                   # [H, L, D]
    # kT pairs: [4, 64, 2L]
    kk = np.transpose(k[0], (2, 0, 1))                  # [64, H, L]
    kT_pack = np.ascontiguousarray(
        np.transpose(kk.reshape(64, 4, 2 * L), (1, 0, 2)))
    # v pairs: [4, 128, 2*VH]
    vv = v[0].reshape(H, L // 128, 128, D).transpose(0, 2, 1, 3).reshape(H, 128, -1)
    VH = vv.shape[2]
    v_pack = np.ascontiguousarray(
        vv.reshape(4, 2, 128, VH).transpose(0, 2, 1, 3).reshape(4, 128, 2 * VH))

    # R = rel_top*g0[k] + rel_bot*g1[k]
    g0 = d_k_score[0, :, 0, 3]
    g1 = d_k_score[0, :, 1, 3]
    R_full = relative_top[0, :, :, 0] * g0[None, :] + relative_bottom[0, :, :, 0] * g1[None, :]

    # coefficient tables
    s0 = d_k_score[0, :, 0, :]                          # [L, 4]
    s1 = d_k_score[0, :, 1, :]
    A3 = (s0 + s1)[:, :3]                               # [L, 3]
    Bv = d_k_top[0] * s0[:, :3] + d_k_bot[0] * s1[:, :3]
    rww = np.empty((H, 4, L), f)
    rwbb = np.empty((H, 4, L), f)
    for h in range(H):
        for fdim in range(3):
            rww[h, fdim] = A3[:, fdim] * W_w[fdim, h]
            rwbb[h, fdim] = A3[:, fdim] * W_b[fdim, h]
        rww[h, 3] = -(Bv @ W_w[:3, h])
        rwbb[h, 3] = -(Bv @ W_b[:3, h])

    uw = np.broadcast_to(W_w[3], (128, H)).copy()
    ub = np.broadcast_to(W_b[3], (128, H)).copy()
    cb = np.broadcast_to(c[0, 0, 0], (128, L)).copy()
    ident = np.eye(128, dtype=f)

    in_maps = []
    for core in range(NCORES):
        qs = qh[:, core * LQ : (core + 1) * LQ, :]      # [H, LQ, D]
        qT = np.ascontiguousarray(np.transpose(qs, (2, 0, 1)).reshape(64, H * LQ))
        Rs = R_full[core * LQ : (core + 1) * LQ]
        Rp = np.ascontiguousarray(Rs.reshape(NROW, 128, L).transpose(1, 0, 2).reshape(128, NROW * L))
        dqT = np.concatenate(
            [d_q[0, core * LQ : (core + 1) * LQ].T, np.ones((1, LQ), f)], axis=0)
        in_maps.append({
            "qt_in": qT, "kt_in": kT_pack, "v_in": v_pack, "r_in": Rp,
            "rww_in": rww, "rwb_in": rwbb, "dqt_in": np.ascontiguousarray(dqT),
            "uw_in": uw, "ub_in": ub, "cb_in": cb, "id_in": ident,
        })
    return in_maps


def kernel(q, k, v, c, d_q, d_k_top, d_k_bot, d_k_score,
           relative_top, relative_bottom, W_w, W_b,
           trace=False):
    global _PROGRAM, LAST_RESULTS
    from concourse import bass_utils

    if _PROGRAM is None:
        _PROGRAM = _build_program()
    nc = _PROGRAM

    in_maps = _host_prep(q, k, v, c, d_q, d_k_top, d_k_bot, d_k_score,
                         relative_top, relative_bottom, W_w, W_b)

    res = bass_utils.run_bass_kernel_spmd(
        nc, in_maps, core_ids=list(range(NCORES)), trace=trace)
    LAST_RESULTS = res

    probs = np.empty((B, H, L, L), np.float32)
    out = np.empty((B, H, L, D), np.float32)
    for core in range(NCORES):
        r = res.results[core]
        probs[0, :, core * LQ : (core + 1) * LQ, :] = r["probs_out"]
        out[0, :, core * LQ : (core + 1) * LQ, :] = np.transpose(r["out_out"], (0, 2, 1))
    return out, probs


# revision 23
# speedup vs baseline: 1.1034x; 1.0841x over previous
"""Trainium2 Bass kernel for sparse_attention (nn_Attention_69965017252614).

Strategy: shard the 2048 query positions across 8 NeuronCores (256 each).
Heavy O(L^2) work stays on device; tiny O(L*F) coefficient tables are
precomputed on host and passed as extra inputs.

Math (per batch, per head h):
  scores = (q/8) @ k^T
  dm[q,k,f<3] = d_q[q,f]*A[k,f] - Bv[k,f]         (rank-4 via augmented matmul)
  dm[q,k,3]   = R[q,k] = rel_top*g0[k] + rel_bot*g1[k]   (dense, head-indep)
  w_pre = dq_aug @ rhsW_h + W_w[3,h]*R ;  b_pre likewise with W_b
  s_fin = scores * softplus(w_pre) + b_pre
  probs = exp(s_fin)/rowsum * c ;  out = probs @ v

Engine assignment per (head, q-tile) row [128 x 2048]:
  PE:   QK matmuls, aug matmuls, probs transposes, PV matmuls
  DVE:  scalar_tensor_tensor (R*u + psum) for w/b, scores*w, reciprocal
  ACT:  softplus via ln(1+exp(x)) (stays in natural_log_exp table set),
        exp with accum_out (free softmax denominator), PSUM->SBUF copies
  Pool: s_fin add, final probs scale (unorm*recip)*c_bcast
"""

import sys
import numpy as np

for _p in ("/opt/trn_rl_repo", "/opt/pypackages"):
    if _p not in sys.path:
        sys.path.insert(0, _p)

B, H, L, D = 1, 8, 2048, 64
NCORES = 8
LQ = L // NCORES          # 256 queries per core
NROW = LQ // 128          # 2 q-tiles of 128 per core
KC = 1024                 # k-chunk for w/b psum tiles

_PROGRAM = None           # (nc, meta) cache — compile once per process
LAST_RESULTS = None       # BassKernelResults from the last run (for test.py)


def _patch_act_tables(bacc, mybir):
    """Make natural_log_exp_and_others the only table set advertising Exp/Ln
    so the act-table-load pass never alternates sets between the softplus
    (exp+ln) and softmax (exp) activations. Set order (= act_func_set_id)
    is preserved; only membership is edited, so the loaded table is still
    the right one."""
    if getattr(bacc, "_act_tables_patched", False):
        return
    orig = bacc.get_activation_tables
    AF = mybir.ActivationFunctionType
    keep = "natural_log_exp_and_others"

    def patched(arch):
        tabs = orig(arch)
        if keep in tabs:
            for name, fns in tabs.items():
                if name != keep:
                    fns.discard(AF.Exp)
                    fns.discard(AF.Ln)
        return tabs

    bacc.get_activation_tables = patched
    bacc._act_tables_patched = True


def _build_program():
    import concourse.bacc as bacc
    import concourse.mybir as mybir
    from concourse.tile import TileContext

    f32 = mybir.dt.float32
    f32r = mybir.dt.float32r
    AF = mybir.ActivationFunctionType
    OP = mybir.AluOpType

    _patch_act_tables(bacc, mybir)
    nc = bacc.Bacc("TRN2", target_bir_lowering=False, debug=False)

    t_qT = nc.dram_tensor("qt_in", (64, H * LQ), f32r, kind="ExternalInput").ap()
    t_kT = nc.dram_tensor("kt_in", (4, 64, 2 * L), f32r, kind="ExternalInput").ap()
    t_v = nc.dram_tensor("v_in", (4, 128, 2 * (L // 128) * D), f32r, kind="ExternalInput").ap()
    t_R = nc.dram_tensor("r_in", (128, NROW * L), f32r, kind="ExternalInput").ap()
    t_rww = nc.dram_tensor("rww_in", (H, 4, L), f32r, kind="ExternalInput").ap()
    t_rwbb = nc.dram_tensor("rwb_in", (H, 4, L), f32r, kind="ExternalInput").ap()
    t_dqT = nc.dram_tensor("dqt_in", (4, LQ), f32r, kind="ExternalInput").ap()
    t_uw = nc.dram_tensor("uw_in", (128, H), f32, kind="ExternalInput").ap()
    t_ub = nc.dram_tensor("ub_in", (128, H), f32, kind="ExternalInput").ap()
    t_cb = nc.dram_tensor("cb_in", (128, L), f32, kind="ExternalInput").ap()
    t_id = nc.dram_tensor("id_in", (128, 128), f32, kind="ExternalInput").ap()

    o_probs = nc.dram_tensor("probs_out", (H, LQ, L), f32, kind="ExternalOutput").ap()
    o_out = nc.dram_tensor("out_out", (H, D, LQ), f32, kind="ExternalOutput").ap()

    VH = (L // 128) * D   # 1024 floats of v per head per partition

    with TileContext(nc) as tc:
        with (
            tc.tile_pool(name="static", bufs=1) as st_pool,
            tc.tile_pool(name="kt", bufs=2) as kt_pool,
            tc.tile_pool(name="vp", bufs=2) as v_pool,
            tc.tile_pool(name="rwb", bufs=1) as rwb_pool,
            tc.tile_pool(name="work", bufs=5) as work_pool,
            tc.tile_pool(name="probs", bufs=4) as pr_pool,
            tc.tile_pool(name="pt", bufs=2) as pt_pool,
            tc.tile_pool(name="small", bufs=4) as sm_pool,
            tc.tile_pool(name="ps_s", bufs=1, space="PSUM") as ps_s_pool,
            tc.tile_pool(name="ps_w", bufs=1, space="PSUM") as ps_w_pool,
            tc.tile_pool(name="ps_b", bufs=1, space="PSUM") as ps_b_pool,
            tc.tile_pool(name="ps_t", bufs=2, space="PSUM") as ps_t_pool,
        ):
            # warm the natural_log_exp act-table while static DMAs stream in
            warm = sm_pool.tile([128, 1], f32, tag="warm")
            nc.gpsimd.memset(warm[:], 0.0)
            nc.scalar.activation(warm[:], warm[:], AF.Exp)
            nc.scalar.activation(warm[:], warm[:], AF.Ln, bias=1.0)

            # ---- static loads, ordered so the first head can start ASAP ------
            qT_sb = st_pool.tile([64, H * LQ], f32r, tag="qT")
            nc.sync.dma_start(qT_sb[:], t_qT[:])
            kt_tiles = {}
            v_tiles = {}
            kt0 = kt_pool.tile([64, L], f32r, tag="kt")
            nc.sync.dma_start(kt0[:], t_kT[0, :, 0:L])
            kt_tiles[0] = kt0
            dqT_sb = st_pool.tile([4, LQ], f32r, tag="dqT")
            nc.sync.dma_start(dqT_sb[:], t_dqT[:])
            uw_sb = st_pool.tile([128, H], f32, tag="uw")
            nc.sync.dma_start(uw_sb[:], t_uw[:])
            ub_sb = st_pool.tile([128, H], f32, tag="ub")
            nc.sync.dma_start(ub_sb[:], t_ub[:])
            id_sb = st_pool.tile([128, 128], f32, tag="ident")
            nc.sync.dma_start(id_sb[:], t_id[:])
            rww0 = rwb_pool.tile([4, L], f32r, tag="rww")
            nc.sync.dma_start(rww0[:], t_rww[0])
            rwbb0 = rwb_pool.tile([4, L], f32r, tag="rwbb")
            nc.sync.dma_start(rwbb0[:], t_rwbb[0])
            R_sb = st_pool.tile([128, NROW * L], f32r, tag="R")
            nc.sync.dma_start(R_sb[:, 0:L], t_R[:, 0:L])
            nc.sync.dma_start(R_sb[:, L : 2 * L], t_R[:, L : 2 * L])
            cb_sb = st_pool.tile([128, L], f32, tag="cb")
            nc.sync.dma_start(cb_sb[:], t_cb[:])

            # deferred-stage state from the previous head
            pend = {}

            def emit_head_tail(ph):
                """Transposes + PV + output stores for head ph (deferred)."""
                p_off = ph % 2
                pv_t = pend["v_t"]
                probsT_h = pt_pool.tile([128, (L // 128) * 256], f32r, tag="pT")
                for qt in range(NROW):
                    probs = pend[("probs", qt)]
                    for g in range(4):
                        ps_t = ps_t_pool.tile([128, 512], f32, tag="T")
                        for j in range(4):
                            kb = g * 4 + j
                            nc.tensor.transpose(
                                ps_t[:, j * 128 : (j + 1) * 128],
                                probs[:, kb * 128 : (kb + 1) * 128],
                                id_sb[:],
                            )
                        dst = probsT_h.rearrange("p (kb q) -> p kb q", q=256)[
                            :, g * 4 : (g + 1) * 4, qt * 128 : qt * 128 + 128]
                        src_ap = ps_t[:].rearrange("p (kb q) -> p kb q", q=128)
                        if g == 0:
                            nc.scalar.copy(dst, src_ap)
                        else:
                            nc.vector.tensor_copy(dst, src_ap)
                ps_oT = ps_t_pool.tile([64, 2 * 128], f32, tag="T")
                nkb = L // 128
                for kb in range(nkb):
                    nc.tensor.matmul(
                        ps_oT[:],
                        pv_t[:, p_off * VH + kb * D : p_off * VH + (kb + 1) * D],
                        probsT_h[:, kb * 256 : (kb + 1) * 256],
                        start=(kb == 0), stop=(kb == nkb - 1),
                    )
                oT_sb = sm_pool.tile([64, 2 * 128], f32, tag="oT")
                nc.scalar.copy(oT_sb[:], ps_oT[:])
                nc.sync.dma_start(o_out[ph], oT_sb[:])

            for h in range(H):
                pair, off = h // 2, h % 2
                if h + 1 < H and (h + 1) // 2 not in kt_tiles:
                    kt_new = kt_pool.tile([64, L], f32r, tag="kt")
                    nc.sync.dma_start(kt_new[:], t_kT[(h + 1) // 2, :, 0:L])
                    kt_tiles[(h + 1) // 2] = kt_new
                if pair not in v_tiles:
                    v_new = v_pool.tile([128, 2 * VH], f32r, tag="vp")
                    nc.sync.dma_start(v_new[:], t_v[pair])
                    v_tiles[pair] = v_new
                # second half of the pair's kT arrives with the odd head
                if off == 0:
                    kt_odd = kt_pool.tile([64, L], f32r, tag="kt")
                    nc.sync.dma_start(kt_odd[:], t_kT[pair, :, L : 2 * L])
                    kt_tiles[pair, 1] = kt_odd
                kt_t = kt_tiles[pair] if off == 0 else kt_tiles[pair, 1]
                v_t = v_tiles[pair]

                uwI = sm_pool.tile([128, 128], f32r, tag="uwI")
                nc.vector.tensor_scalar_mul(uwI[:], id_sb[:], uw_sb[:, h : h + 1])
                ubI = sm_pool.tile([128, 128], f32r, tag="ubI")
                nc.vector.tensor_scalar_mul(ubI[:], id_sb[:], ub_sb[:, h : h + 1])
                if h == 0:
                    rww_t, rwbb_t = rww0, rwbb0
                else:
                    rww_t = rwb_pool.tile([4, L], f32r, tag="rww")
                    nc.sync.dma_start(rww_t[:], t_rww[h])
                    rwbb_t = rwb_pool.tile([4, L], f32r, tag="rwbb")
                    nc.sync.dma_start(rwbb_t[:], t_rwbb[h])

                new_pend = {"v_t": v_t}
                for qt in range(NROW):
                    qsl = qT_sb[:, h * LQ + qt * 128 : h * LQ + qt * 128 + 128]
                    dqsl = dqT_sb[:, qt * 128 : qt * 128 + 128]
                    Rrow = R_sb[:, qt * L : (qt + 1) * L]

                    # --- scores: two [128, 1024] psum tiles -------------------
                    ps_s_half = []
                    for cs in range(2):
                        ps_s = ps_s_pool.tile([128, 1024], f32, tag="s")
                        ps_s_half.append(ps_s)
                        for n in range(2):
                            col = cs * 1024 + n * 512
                            nc.tensor.matmul(
                                ps_s[:, n * 512 : (n + 1) * 512],
                                qsl,
                                kt_t[:, col : col + 512],
                                start=True,
                                stop=True,
                            )

                    # --- w/b aug matmuls; u*R accumulated via f32r identity ---
                    w_row = work_pool.tile([128, L], f32, tag="work")
                    sw_row = work_pool.tile([128, L], f32, tag="work")
                    sfin = work_pool.tile([128, L], f32, tag="work")
                    denom = sm_pool.tile([128, 2], f32, tag="denom")
                    for cs in range(2):
                        sl2 = slice(cs * 1024, (cs + 1) * 1024)
                        ps_w = ps_w_pool.tile([128, 1024], f32, tag="w")
                        ps_b = ps_b_pool.tile([128, 1024], f32, tag="b")
                        for n in range(2):
                            ssl = slice(cs * 1024 + n * 512, cs * 1024 + (n + 1) * 512)
                            nsl = slice(n * 512, (n + 1) * 512)
                            nc.tensor.matmul(ps_w[:, nsl], dqsl, rww_t[:, ssl],
                                             start=True, stop=False)
                            nc.tensor.matmul(ps_w[:, nsl], uwI[:], Rrow[:, ssl],
                                             start=False, stop=True)
                            nc.tensor.matmul(ps_b[:, nsl], dqsl, rwbb_t[:, ssl],
                                             start=True, stop=False)
                            nc.tensor.matmul(ps_b[:, nsl], ubI[:], Rrow[:, ssl],
                                             start=False, stop=True)
                        # softplus from psum: w = ln(1 + exp(ps_w))
                        nc.scalar.activation(w_row[:, sl2], ps_w[:], AF.Exp)
                        nc.scalar.activation(w_row[:, sl2], w_row[:, sl2], AF.Ln, bias=1.0)
                        nc.vector.tensor_tensor(sw_row[:, sl2], ps_s_half[cs][:],
                                                w_row[:, sl2], op=OP.mult)
                        nc.vector.tensor_tensor(sfin[:, sl2], sw_row[:, sl2],
                                                ps_b[:], op=OP.add)
                        nc.scalar.activation(sfin[:, sl2], sfin[:, sl2], AF.Exp,
                                             accum_out=denom[:, cs : cs + 1])
                    unorm = sfin
                    recip = sm_pool.tile([128, 1], f32, tag="recip")
                    dsum = sm_pool.tile([128, 1], f32, tag="dsum")
                    nc.vector.tensor_tensor(dsum[:], denom[:, 0:1], denom[:, 1:2], op=OP.add)
                    nc.vector.reciprocal(recip[:], dsum[:])

                    # --- probs: t2 = unorm*c (Pool), probs = t2*recip (DVE) ---
                    probs = pr_pool.tile([128, L], f32, tag="probs")
                    t2 = work_pool.tile([128, L], f32, tag="work")
                    for cs in range(2):
                        sl2 = slice(cs * 1024, (cs + 1) * 1024)
                        nc.gpsimd.tensor_tensor(t2[:, sl2], unorm[:, sl2],
                                                cb_sb[:, sl2], op=OP.mult)
                        nc.vector.tensor_scalar_mul(probs[:, sl2], t2[:, sl2], recip[:])
                    dma_eng = nc.sync if (h + qt) % 2 == 0 else nc.gpsimd
                    dma_eng.dma_start(o_probs[h, qt * 128 : (qt + 1) * 128, :], probs[:])
                    new_pend[("probs", qt)] = probs

                if pend:
                    emit_head_tail(h - 1)
                pend = new_pend
            emit_head_tail(H - 1)

    nc.compile()
    return nc


def _host_prep(q, k, v, c, d_q, d_k_top, d_k_bot, d_k_score,
               relative_top, relative_bottom, W_w, W_b):
    """Build the per-core input maps (all fp32 numpy)."""
    f = np.float32
    q = np.asarray(q, f); k = np.asarray(k, f); v = np.asarray(v, f)
    c = np.asarray(c, f); d_q = np.asarray(d_q, f)
    d_k_top = np.asarray(d_k_top, f); d_k_bot = np.asarray(d_k_bot, f)
    d_k_score = np.asarray(d_k_score, f)
    relative_top = np.asarray(relative_top, f)
    relative_bottom = np.asarray(relative_bottom, f)
    W_w = np.asarray(W_w, f); W_b = np.asarray(W_b, f)

    scale = f(1.0 / np.sqrt(D))

    # qT per core: [64, H*LQ], h-major columns
    qh = q[0] * scale                                   # [H, L, D]
    # kT pairs: [4, 64, 2L]
    kk = np.transpose(k[0], (2, 0, 1))                  # [64, H, L]
    kT_pack = np.ascontiguousarray(
        np.transpose(kk.reshape(64, 4, 2 * L), (1, 0, 2)))
    # v pairs: [4, 128, 2*VH]
    vv = v[0].reshape(H, L // 128, 128, D).transpose(0, 2, 1, 3).reshape(H, 128, -1)
    VH = vv.shape[2]
    v_pack = np.ascontiguousarray(
        vv.reshape(4, 2, 128, VH).transpose(0, 2, 1, 3).reshape(4, 128, 2 * VH))

    # R = rel_top*g0[k] + rel_bot*g1[k]
    g0 = d_k_score[0, :, 0, 3]
    g1 = d_k_score[0, :, 1, 3]
    R_full = relative_top[0, :, :, 0] * g0[None, :] + relative_bottom[0, :, :, 0] * g1[None, :]

    # coefficient tables
    s0 = d_k_score[0, :, 0, :]                          # [L, 4]
    s1 = d_k_score[0, :, 1, :]
    A3 = (s0 + s1)[:, :3]                               # [L, 3]
    Bv = d_k_top[0] * s0[:, :3] + d_k_bot[0] * s1[:, :3]
    rww = np.empty((H, 4, L), f)
    rwbb = np.empty((H, 4, L), f)
    for h in range(H):
        for fdim in range(3):
            rww[h, fdim] = A3[:, fdim] * W_w[fdim, h]
            rwbb[h, fdim] = A3[:, fdim] * W_b[fdim, h]
        rww[h, 3] = -(Bv @ W_w[:3, h])
        rwbb[h, 3] = -(Bv @ W_b[:3, h])

    uw = np.broadcast_to(W_w[3], (128, H)).copy()
    ub = np.broadcast_to(W_b[3], (128, H)).copy()
    cb = np.broadcast_to(c[0, 0, 0], (128, L)).copy()
    ident = np.eye(128, dtype=f)

    in_maps = []
    for core in range(NCORES):
        qs = qh[:, core * LQ : (core + 1) * LQ, :]      # [H, LQ, D]
        qT = np.ascontiguousarray(np.transpose(qs, (2, 0, 1)).reshape(64, H * LQ))
        Rs = R_full[core * LQ : (core + 1) * LQ]
        Rp = np.ascontiguousarray(Rs.reshape(NROW, 128, L).transpose(1, 0, 2).reshape(128, NROW * L))
        dqT = np.concatenate(
            [d_q[0, core * LQ : (core + 1) * LQ].T, np.ones((1, LQ), f)], axis=0)
        in_maps.append({
            "qt_in": qT, "kt_in": kT_pack, "v_in": v_pack, "r_in": Rp,
            "rww_in": rww, "rwb_in": rwbb, "dqt_in": np.ascontiguousarray(dqT),
            "uw_in": uw, "ub_in": ub, "cb_in": cb, "id_in": ident,
        })
    return in_maps


def kernel(q, k, v, c, d_q, d_k_top, d_k_bot, d_k_score,
           relative_top, relative_bottom, W_w, W_b,
           trace=False):
    global _PROGRAM, LAST_RESULTS
    from concourse import bass_utils

    if _PROGRAM is None:
        _PROGRAM = _build_program()
    nc = _PROGRAM

    in_maps = _host_prep(q, k, v, c, d_q, d_k_top, d_k_bot, d_k_score,
                         relative_top, relative_bottom, W_w, W_b)

    res = bass_utils.run_bass_kernel_spmd(
        nc, in_maps, core_ids=list(range(NCORES)), trace=trace)
    LAST_RESULTS = res

    probs = np.empty((B, H, L, L), np.float32)
    out = np.empty((B, H, L, D), np.float32)
    for core in range(NCORES):
        r = res.results[core]
        probs[0, :, core * LQ : (core + 1) * LQ, :] = r["probs_out"]
        out[0, :, core * LQ : (core + 1) * LQ, :] = np.transpose(r["out_out"], (0, 2, 1))
    return out, probs


# revision 24
# speedup vs baseline: 1.1865x; 1.0753x over previous
"""Trainium2 Bass kernel for sparse_attention (nn_Attention_69965017252614).

Strategy: shard the 2048 query positions across 8 NeuronCores (256 each).
Heavy O(L^2) work stays on device; tiny O(L*F) coefficient tables are
precomputed on host and passed as extra inputs.

Math (per batch, per head h):
  scores = (q/8) @ k^T
  dm[q,k,f<3] = d_q[q,f]*A[k,f] - Bv[k,f]         (rank-4 via augmented matmul)
  dm[q,k,3]   = R[q,k] = rel_top*g0[k] + rel_bot*g1[k]   (dense, head-indep)
  w_pre = dq_aug @ rhsW_h + W_w[3,h]*R ;  b_pre likewise with W_b
  s_fin = scores * softplus(w_pre) + b_pre
  probs = exp(s_fin)/rowsum * c ;  out = probs @ v

Engine assignment per (head, q-tile) row [128 x 2048]:
  PE:   QK matmuls, aug matmuls, probs transposes, PV matmuls
  DVE:  scalar_tensor_tensor (R*u + psum) for w/b, scores*w, reciprocal
  ACT:  softplus via ln(1+exp(x)) (stays in natural_log_exp table set),
        exp with accum_out (free softmax denominator), PSUM->SBUF copies
  Pool: s_fin add, final probs scale (unorm*recip)*c_bcast
"""

import sys
import numpy as np

for _p in ("/opt/trn_rl_repo", "/opt/pypackages"):
    if _p not in sys.path:
        sys.path.insert(0, _p)

B, H, L, D = 1, 8, 2048, 64
NCORES = 8
LQ = L // NCORES          # 256 queries per core
NROW = LQ // 128          # 2 q-tiles of 128 per core
KC = 1024                 # k-chunk for w/b psum tiles

_PROGRAM = None           # (nc, meta) cache — compile once per process
LAST_RESULTS = None       # BassKernelResults from the last run (for test.py)


def _patch_act_tables(bacc, mybir):
    """Make natural_log_exp_and_others the only table set advertising Exp/Ln
    so the act-table-load pass never alternates sets between the softplus
    (exp+ln) and softmax (exp) activations. Set order (= act_func_set_id)
    is preserved; only membership is edited, so the loaded table is still
    the right one."""
    if getattr(bacc, "_act_tables_patched", False):
        return
    orig = bacc.get_activation_tables
    AF = mybir.ActivationFunctionType
    keep = "natural_log_exp_and_others"

    def patched(arch):
        tabs = orig(arch)
        if keep in tabs:
            for name, fns in tabs.items():
                if name != keep:
                    fns.discard(AF.Exp)
                    fns.discard(AF.Ln)
        return tabs

    bacc.get_activation_tables = patched
    bacc._act_tables_patched = True


def _build_program():
    import concourse.bacc as bacc
    import concourse.mybir as mybir
    from concourse.tile import TileContext

    f32 = mybir.dt.float32
    f32r = mybir.dt.float32r
    AF = mybir.ActivationFunctionType
    OP = mybir.AluOpType

    _patch_act_tables(bacc, mybir)
    nc = bacc.Bacc("TRN2", target_bir_lowering=False, debug=False)

    t_qT = nc.dram_tensor("qt_in", (64, H * LQ), f32r, kind="ExternalInput").ap()
    t_kT = nc.dram_tensor("kt_in", (4, 64, 2 * L), f32r, kind="ExternalInput").ap()
    t_v = nc.dram_tensor("v_in", (4, 128, 2 * (L // 128) * D), f32r, kind="ExternalInput").ap()
    t_R = nc.dram_tensor("r_in", (128, NROW * L), f32r, kind="ExternalInput").ap()
    t_rww = nc.dram_tensor("rww_in", (H, 4, L), f32r, kind="ExternalInput").ap()
    t_rwbb = nc.dram_tensor("rwb_in", (H, 4, L), f32r, kind="ExternalInput").ap()
    t_dqT = nc.dram_tensor("dqt_in", (4, LQ), f32r, kind="ExternalInput").ap()
    t_uw = nc.dram_tensor("uw_in", (128, H), f32, kind="ExternalInput").ap()
    t_ub = nc.dram_tensor("ub_in", (128, H), f32, kind="ExternalInput").ap()
    t_cb = nc.dram_tensor("cb_in", (128, L), f32, kind="ExternalInput").ap()
    t_id = nc.dram_tensor("id_in", (128, 128), f32, kind="ExternalInput").ap()

    o_probs = nc.dram_tensor("probs_out", (H, LQ, L), f32, kind="ExternalOutput").ap()
    o_out = nc.dram_tensor("out_out", (H, D, LQ), f32, kind="ExternalOutput").ap()

    VH = (L // 128) * D   # 1024 floats of v per head per partition

    with TileContext(nc) as tc:
        with (
            tc.tile_pool(name="static", bufs=1) as st_pool,
            tc.tile_pool(name="kt", bufs=2) as kt_pool,
            tc.tile_pool(name="vp", bufs=2) as v_pool,
            tc.tile_pool(name="rwb", bufs=1) as rwb_pool,
            tc.tile_pool(name="work", bufs=5) as work_pool,
            tc.tile_pool(name="probs", bufs=4) as pr_pool,
            tc.tile_pool(name="pt", bufs=2) as pt_pool,
            tc.tile_pool(name="small", bufs=4) as sm_pool,
            tc.tile_pool(name="ps_s", bufs=1, space="PSUM") as ps_s_pool,
            tc.tile_pool(name="ps_w", bufs=1, space="PSUM") as ps_w_pool,
            tc.tile_pool(name="ps_b", bufs=1, space="PSUM") as ps_b_pool,
            tc.tile_pool(name="ps_t", bufs=2, space="PSUM") as ps_t_pool,
        ):
            # warm the natural_log_exp act-table while static DMAs stream in
            warm = sm_pool.tile([128, 1], f32, tag="warm")
            nc.gpsimd.memset(warm[:], 0.0)
            nc.scalar.activation(warm[:], warm[:], AF.Exp)
            nc.scalar.activation(warm[:], warm[:], AF.Ln, bias=1.0)

            # ---- static loads, ordered so the first head can start ASAP ------
            qT_sb = st_pool.tile([64, H * LQ], f32r, tag="qT")
            nc.sync.dma_start(qT_sb[:], t_qT[:])
            kt_tiles = {}
            v_tiles = {}
            kt0 = kt_pool.tile([64, L], f32r, tag="kt")
            nc.sync.dma_start(kt0[:], t_kT[0, :, 0:L])
            kt_tiles[0] = kt0
            dqT_sb = st_pool.tile([4, LQ], f32r, tag="dqT")
            nc.sync.dma_start(dqT_sb[:], t_dqT[:])
            uw_sb = st_pool.tile([128, H], f32, tag="uw")
            nc.sync.dma_start(uw_sb[:], t_uw[:])
            ub_sb = st_pool.tile([128, H], f32, tag="ub")
            nc.sync.dma_start(ub_sb[:], t_ub[:])
            id_sb = st_pool.tile([128, 128], f32, tag="ident")
            nc.sync.dma_start(id_sb[:], t_id[:])
            rww0 = rwb_pool.tile([4, L], f32r, tag="rww")
            nc.sync.dma_start(rww0[:], t_rww[0])
            rwbb0 = rwb_pool.tile([4, L], f32r, tag="rwbb")
            nc.sync.dma_start(rwbb0[:], t_rwbb[0])
            R_sb = st_pool.tile([128, NROW * L], f32r, tag="R")
            nc.sync.dma_start(R_sb[:, 0:L], t_R[:, 0:L])
            nc.sync.dma_start(R_sb[:, L : 2 * L], t_R[:, L : 2 * L])
            cb_sb = st_pool.tile([128, L], f32, tag="cb")
            nc.sync.dma_start(cb_sb[:], t_cb[:])

            # deferred-stage state from the previous head
            pend = {}

            def emit_head_tail(ph):
                """Transposes + PV + output stores for head ph (deferred)."""
                p_off = ph % 2
                pv_t = pend["v_t"]
                probsT_h = pt_pool.tile([128, (L // 128) * 256], f32r, tag="pT")
                for qt in range(NROW):
                    probs = pend[("probs", qt)]
                    for g in range(4):
                        ps_t = ps_t_pool.tile([128, 512], f32, tag="T")
                        for j in range(4):
                            kb = g * 4 + j
                            nc.tensor.transpose(
                                ps_t[:, j * 128 : (j + 1) * 128],
                                probs[:, kb * 128 : (kb + 1) * 128],
                                id_sb[:],
                            )
                        dst = probsT_h.rearrange("p (kb q) -> p kb q", q=256)[
                            :, g * 4 : (g + 1) * 4, qt * 128 : qt * 128 + 128]
                        src_ap = ps_t[:].rearrange("p (kb q) -> p kb q", q=128)
                        if g == 0:
                            nc.scalar.copy(dst, src_ap)
                        else:
                            nc.vector.tensor_copy(dst, src_ap)
                ps_oT = ps_t_pool.tile([64, 2 * 128], f32, tag="T")
                nkb = L // 128
                for kb in range(nkb):
                    nc.tensor.matmul(
                        ps_oT[:],
                        pv_t[:, p_off * VH + kb * D : p_off * VH + (kb + 1) * D],
                        probsT_h[:, kb * 256 : (kb + 1) * 256],
                        start=(kb == 0), stop=(kb == nkb - 1),
                    )
                oT_sb = sm_pool.tile([64, 2 * 128], f32, tag="oT")
                nc.scalar.copy(oT_sb[:], ps_oT[:])
                nc.sync.dma_start(o_out[ph], oT_sb[:])

            for h in range(H):
                pair, off = h // 2, h % 2
                if h + 1 < H and (h + 1) // 2 not in kt_tiles:
                    kt_new = kt_pool.tile([64, L], f32r, tag="kt")
                    nc.gpsimd.dma_start(kt_new[:], t_kT[(h + 1) // 2, :, 0:L])
                    kt_tiles[(h + 1) // 2] = kt_new
                if pair not in v_tiles:
                    v_new = v_pool.tile([128, 2 * VH], f32r, tag="vp")
                    nc.gpsimd.dma_start(v_new[:], t_v[pair])
                    v_tiles[pair] = v_new
                # second half of the pair's kT arrives with the odd head
                if off == 0:
                    kt_odd = kt_pool.tile([64, L], f32r, tag="kt")
                    eng = nc.sync if h == 0 else nc.gpsimd
                    eng.dma_start(kt_odd[:], t_kT[pair, :, L : 2 * L])
                    kt_tiles[pair, 1] = kt_odd
                kt_t = kt_tiles[pair] if off == 0 else kt_tiles[pair, 1]
                v_t = v_tiles[pair]

                uwI = sm_pool.tile([128, 128], f32r, tag="uwI")
                nc.vector.tensor_scalar_mul(uwI[:], id_sb[:], uw_sb[:, h : h + 1])
                ubI = sm_pool.tile([128, 128], f32r, tag="ubI")
                nc.vector.tensor_scalar_mul(ubI[:], id_sb[:], ub_sb[:, h : h + 1])
                if h == 0:
                    rww_t, rwbb_t = rww0, rwbb0
                else:
                    rww_t = rwb_pool.tile([4, L], f32r, tag="rww")
                    nc.sync.dma_start(rww_t[:], t_rww[h])
                    rwbb_t = rwb_pool.tile([4, L], f32r, tag="rwbb")
                    nc.sync.dma_start(rwbb_t[:], t_rwbb[h])

                new_pend = {"v_t": v_t}
                for qt in range(NROW):
                    qsl = qT_sb[:, h * LQ + qt * 128 : h * LQ + qt * 128 + 128]
                    dqsl = dqT_sb[:, qt * 128 : qt * 128 + 128]
                    Rrow = R_sb[:, qt * L : (qt + 1) * L]

                    # --- scores: two [128, 1024] psum tiles -------------------
                    ps_s_half = []
                    for cs in range(2):
                        ps_s = ps_s_pool.tile([128, 1024], f32, tag="s")
                        ps_s_half.append(ps_s)
                        for n in range(2):
                            col = cs * 1024 + n * 512
                            nc.tensor.matmul(
                                ps_s[:, n * 512 : (n + 1) * 512],
                                qsl,
                                kt_t[:, col : col + 512],
                                start=True,
                                stop=True,
                            )

                    # --- w/b aug matmuls; u*R accumulated via f32r identity ---
                    w_row = work_pool.tile([128, L], f32, tag="work")
                    sw_row = work_pool.tile([128, L], f32, tag="work")
                    sfin = work_pool.tile([128, L], f32, tag="work")
                    denom = sm_pool.tile([128, 2], f32, tag="denom")
                    for cs in range(2):
                        sl2 = slice(cs * 1024, (cs + 1) * 1024)
                        ps_w = ps_w_pool.tile([128, 1024], f32, tag="w")
                        ps_b = ps_b_pool.tile([128, 1024], f32, tag="b")
                        for n in range(2):
                            ssl = slice(cs * 1024 + n * 512, cs * 1024 + (n + 1) * 512)
                            nsl = slice(n * 512, (n + 1) * 512)
                            nc.tensor.matmul(ps_w[:, nsl], dqsl, rww_t[:, ssl],
                                             start=True, stop=False)
                            nc.tensor.matmul(ps_w[:, nsl], uwI[:], Rrow[:, ssl],
                                             start=False, stop=True)
                            nc.tensor.matmul(ps_b[:, nsl], dqsl, rwbb_t[:, ssl],
                                             start=True, stop=False)
                            nc.tensor.matmul(ps_b[:, nsl], ubI[:], Rrow[:, ssl],
                                             start=False, stop=True)
                        # softplus from psum: w = ln(1 + exp(ps_w))
                        nc.scalar.activation(w_row[:, sl2], ps_w[:], AF.Exp)
                        nc.scalar.activation(w_row[:, sl2], w_row[:, sl2], AF.Ln, bias=1.0)
                        nc.vector.tensor_tensor(sw_row[:, sl2], ps_s_half[cs][:],
                                                w_row[:, sl2], op=OP.mult)
                        nc.vector.tensor_tensor(sfin[:, sl2], sw_row[:, sl2],
                                                ps_b[:], op=OP.add)
                        nc.scalar.activation(sfin[:, sl2], sfin[:, sl2], AF.Exp,
                                             accum_out=denom[:, cs : cs + 1])
                    unorm = sfin
                    recip = sm_pool.tile([128, 1], f32, tag="recip")
                    dsum = sm_pool.tile([128, 1], f32, tag="dsum")
                    nc.vector.tensor_tensor(dsum[:], denom[:, 0:1], denom[:, 1:2], op=OP.add)
                    nc.vector.reciprocal(recip[:], dsum[:])

                    # --- probs: t2 = unorm*c (Pool), probs = t2*recip (DVE) ---
                    probs = pr_pool.tile([128, L], f32, tag="probs")
                    t2 = work_pool.tile([128, L], f32, tag="work")
                    for cs in range(2):
                        sl2 = slice(cs * 1024, (cs + 1) * 1024)
                        nc.gpsimd.tensor_tensor(t2[:, sl2], unorm[:, sl2],
                                                cb_sb[:, sl2], op=OP.mult)
                        nc.vector.tensor_scalar_mul(probs[:, sl2], t2[:, sl2], recip[:])
                    dma_eng = nc.sync if (h + qt) % 2 == 0 else nc.gpsimd
                    dma_eng.dma_start(o_probs[h, qt * 128 : (qt + 1) * 128, :], probs[:])
                    new_pend[("probs", qt)] = probs

                if pend:
                    emit_head_tail(h - 1)
                pend = new_pend
            emit_head_tail(H - 1)

    nc.compile()
    return nc


def _host_prep(q, k, v, c, d_q, d_k_top, d_k_bot, d_k_score,
               relative_top, relative_bottom, W_w, W_b):
    """Build the per-core input maps (all fp32 numpy)."""
    f = np.float32
    q = np.asarray(q, f); k = np.asarray(k, f); v = np.asarray(v, f)
    c = np.asarray(c, f); d_q = np.asarray(d_q, f)
    d_k_top = np.asarray(d_k_top, f); d_k_bot = np.asarray(d_k_bot, f)
    d_k_score = np.asarray(d_k_score, f)
    relative_top = np.asarray(relative_top, f)
    relative_bottom = np.asarray(relative_bottom, f)
    W_w = np.asarray(W_w, f); W_b = np.asarray(W_b, f)

    scale = f(1.0 / np.sqrt(D))

    # qT per core: [64, H*LQ], h-major columns
    qh = q[0] * scale                                   # [H, L, D]
    # kT pairs: [4, 64, 2L]
    kk = np.transpose(k[0], (2, 0, 1))                  # [64, H, L]
    kT_pack = np.ascontiguousarray(
        np.transpose(kk.reshape(64, 4, 2 * L), (1, 0, 2)))
    # v pairs: [4, 128, 2*VH]
    vv = v[0].reshape(H, L // 128, 128, D).transpose(0, 2, 1, 3).reshape(H, 128, -1)
    VH = vv.shape[2]
    v_pack = np.ascontiguousarray(
        vv.reshape(4, 2, 128, VH).transpose(0, 2, 1, 3).reshape(4, 128, 2 * VH))

    # R = rel_top*g0[k] + rel_bot*g1[k]
    g0 = d_k_score[0, :, 0, 3]
    g1 = d_k_score[0, :, 1, 3]
    R_full = relative_top[0, :, :, 0] * g0[None, :] + relative_bottom[0, :, :, 0] * g1[None, :]

    # coefficient tables
    s0 = d_k_score[0, :, 0, :]                          # [L, 4]
    s1 = d_k_score[0, :, 1, :]
    A3 = (s0 + s1)[:, :3]                               # [L, 3]
    Bv = d_k_top[0] * s0[:, :3] + d_k_bot[0] * s1[:, :3]
    rww = np.empty((H, 4, L), f)
    rwbb = np.empty((H, 4, L), f)
    for h in range(H):
        for fdim in range(3):
            rww[h, fdim] = A3[:, fdim] * W_w[fdim, h]
            rwbb[h, fdim] = A3[:, fdim] * W_b[fdim, h]
        rww[h, 3] = -(Bv @ W_w[:3, h])
        rwbb[h, 3] = -(Bv @ W_b[:3, h])

    uw = np.broadcast_to(W_w[3], (128, H)).copy()
    ub = np.broadcast_to(W_b[3], (128, H)).copy()
    cb = np.broadcast_to(c[0, 0, 0], (128, L)).copy()
    ident = np.eye(128, dtype=f)

    in_maps = []
    for core in range(NCORES):
        qs = qh[:, core * LQ : (core + 1) * LQ, :]      # [H, LQ, D]
        qT = np.ascontiguousarray(np.transpose(qs, (2, 0, 1)).reshape(64, H * LQ))
        Rs = R_full[core * LQ : (core + 1) * LQ]
        Rp = np.ascontiguousarray(Rs.reshape(NROW, 128, L).transpose(1, 0, 2).reshape(128, NROW * L))
        dqT = np.concatenate(
            [d_q[0, core * LQ : (core + 1) * LQ].T, np.ones((1, LQ), f)], axis=0)
        in_maps.append({
            "qt_in": qT, "kt_in": kT_pack, "v_in": v_pack, "r_in": Rp,
            "rww_in": rww, "rwb_in": rwbb, "dqt_in": np.ascontiguousarray(dqT),
            "uw_in": uw, "ub_in": ub, "cb_in": cb, "id_in": ident,
        })
    return in_maps


def kernel(q, k, v, c, d_q, d_k_top, d_k_bot, d_k_score,
           relative_top, relative_bottom, W_w, W_b,
           trace=False):
    global _PROGRAM, LAST_RESULTS
    from concourse import bass_utils

    if _PROGRAM is None:
        _PROGRAM = _build_program()
    nc = _PROGRAM

    in_maps = _host_prep(q, k, v, c, d_q, d_k_top, d_k_bot, d_k_score,
                         relative_top, relative_bottom, W_w, W_b)

    res = bass_utils.run_bass_kernel_spmd(
        nc, in_maps, core_ids=list(range(NCORES)), trace=trace)
    LAST_RESULTS = res

    probs = np.empty((B, H, L, L), np.float32)
    out = np.empty((B, H, L, D), np.float32)
    for core in range(NCORES):
        r = res.results[core]
        probs[0, :, core * LQ : (core + 1) * LQ, :] = r["probs_out"]
        out[0, :, core * LQ : (core + 1) * LQ, :] = np.transpose(r["out_out"], (0, 2, 1))
    return out, probs
